# revision 56
# baseline (speedup 1.0000x reference)
"""Trainium2 Bass kernel for CausalAttentiveStatisticsPooling.

Problem (per batch element b, data-parallel over 8 cores):
  mask_t   = t < L_b
  c_mean   = cumsum(x*mask)/count,  c_std = sqrt(clip(cumsum(x^2*mask)/count - c_mean^2, eps))
  h        = tanh(w1^T [x; c_mean; c_std] + b1);  scores = w2^T h + b2   (per key t)
  attn(i,j)= softmax_j<=i(scores_j * valid_j)     -> collapses to e_j / E_i with
             e = exp(scores)*mask, E = cumsum(e)
  w_mean_i = R_i * cumsum(e*x)_i,   w_var_i = R_i * cumsum(e*x^2)_i - w_mean_i^2
  out      = [sum_i<L w_mean_i / L, sum_i<L sqrt(clip(w_var,eps)) / L]    (2C,)

Layout: T=2048 on partitions as 16 chunks of 128; x and x^2 chunks adjacent in
the SBUF free dim (one [128,2,C] tile pair per chunk).  Cumsums along T =
per-chunk upper-triangular matmuls + cross-chunk carries via a chunk-totals
matrix and host-static cumulative-select lhsT matmuls.  Bulk matmul data
(x, x^2, xN, w1, triu/onescols/ident) is bf16 (full-rate, halves DMA);
carries/selects run f32r; PSUM accumulation is fp32 throughout.

Software pipelining against the in-order engine queues:
  phase 1 (per chunk i): [PE] triu+carry mms -> [ACT] sq=(S1*rcnt)^2 (Square
  w/ scale, PSUM read) -> [DVE] var=stt(S2,rcnt,sq) -> [Pool] clip -> bf16.
  The 3C->A MLP is split into per-cb units spread one per iteration (lag 3):
  [PE] 4 transposes -> [ACT] sqrt fused into the PSUM eviction -> [PE] w1s
  matmul; c_mean path is project-then-scan (no transposes).  The last group
  pre-sqrts in the chunk chain so the tanh act-table switch happens early.
  tanh/scores/exp are deferred after all sqrts (act-table grouping).
  phase 2 (per chunk, stage-split, finals lag): [PE] wtri+carry mms ->
  [ACT] wm=mp*R (Copy w/ scale) -> [DVE] sq2=wm^2, var2=stt(ap,R,sq2) ->
  [Pool/ACT] clip -> [ACT] ws=sqrt(+eps) -> [PE] finals matmuls.
  PSUM: phase PSUM tiles rotate through 3 slots (2 x sA + the idle totals
  bank) so the elementwise chains never stall the tensor engine.
"""

import numpy as np
import ml_dtypes

B, C, T, A = 8, 512, 2048, 128
NCH = T // 128  # 16 T-chunks
EPS = 1e-12
NEG = -30000.0

_CACHE = {}


def _build():
    import concourse.bass as bass
    import concourse.mybir as mybir
    import concourse.tile as tile
    from concourse.tile import add_dep_helper
    from concourse import bacc

    f32 = mybir.dt.float32
    f32r = mybir.dt.float32r
    bf16 = mybir.dt.bfloat16
    AF = mybir.ActivationFunctionType
    OP = mybir.AluOpType

    nc = bacc.Bacc("TRN2", target_bir_lowering=False, debug=False,
                   num_devices=8)

    def din(name, shape, dt=f32r):
        return nc.dram_tensor(name, shape, dt, kind="ExternalInput").ap()

    d_xT = din("xT", (T, C), bf16)
    d_xN = din("xN", (C, T), bf16)
    d_w1 = din("w1", (3 * C, A), bf16)
    d_rcntb = din("rcntb", (128, T), bf16)
    # one blob for all small consts: triu|onescols|ident|sutri16r|sutri16|
    # onesc|rcnt|maskexp|finalw|w2|b1  (f32 entries bitcast on use)
    NB = 128 + 256 + 128 + 16 + 16 + 1 + 16 + 16 + 16 + 1 + 1
    d_cblob = din("cblob", (128, NB))
    # bf16 twin of the matmul constants: triu|onescols|ident|finalw
    NBB = 128 + 256 + 128 + 16
    d_bblob = din("bblob", (128, NBB), bf16)
    d_out = nc.dram_tensor("out", (1, 2 * C), f32, kind="ExternalOutput").ap()

    NSPLIT = 8
    FIN_LAG = 3   # finals for chunk k run during phase-2 iteration k + FIN_LAG

    with tile.TileContext(nc) as tc:
        with (
            tc.tile_pool(name="big", bufs=1) as big,
            tc.tile_pool(name="consts", bufs=1) as consts,
            tc.tile_pool(name="csp", bufs=10) as csp_pool,
            tc.tile_pool(name="tmp", bufs=3) as tmp,
            tc.tile_pool(name="wws", bufs=FIN_LAG + 1) as wws_pool,
            tc.tile_pool(name="wsp", bufs=1) as wsp,
            tc.tile_pool(name="hp", bufs=4) as hp,
            tc.tile_pool(name="natp", bufs=4) as natp,
            tc.tile_pool(name="colp", bufs=1) as colp,
            tc.tile_pool(name="ps_s", bufs=2, space="PSUM") as ps_s,
            tc.tile_pool(name="ps_tot", bufs=1, space="PSUM") as ps_tot,
            tc.tile_pool(name="ps_col", bufs=1, space="PSUM") as ps_col,
        ):
            # ---------------- load inputs ----------------
            # bf16 matmul consts + first xT chunks go first (first matmuls
            # need only tb_onescols + xx[0]); the f32 blob follows.
            t_bblob = consts.tile([128, NBB], bf16)
            nc.sync.dma_start(t_bblob, d_bblob)
            tb_triu = t_bblob[:, 0:128]
            tb_onescols = t_bblob[:, 128:384]
            tb_ident = t_bblob[:, 384:512]
            tb_finalw = t_bblob[:, 512:528]

            xx = big.tile([128, NCH, 2, C], bf16)
            d_xT_r = d_xT.rearrange("(n p) c -> p n c", p=128)
            for i in range(NCH):
                if i == NSPLIT:
                    break
                nc.sync.dma_start(xx[:, i, 0, :], d_xT_r[:, i, :])
                nc.vector.tensor_mul(xx[:, i, 1, :], xx[:, i, 0, :],
                                     xx[:, i, 0, :])

            t_cblob = consts.tile([128, NB], f32r)
            nc.sync.dma_start(t_cblob, d_cblob)
            o = [0]
            def blob(n, dt=None):
                sl = t_cblob[:, o[0]:o[0] + n]
                o[0] += n
                return sl.bitcast(dt) if dt is not None else sl
            t_triu = blob(128)
            t_onescols = blob(256)
            t_ident = blob(128)
            t_sutri16r_full = blob(16)
            t_sutri16 = blob(16, f32)[0:16, :]
            t_onesc = blob(1, f32)
            t_rcnt = blob(16, f32)
            t_maskexp = blob(16, f32)
            t_finalw = blob(16)
            t_w2 = blob(1, f32)[0:A, :]
            t_b1 = blob(1, f32)[0:A, :]
            t_sutri16r = t_sutri16r_full[0:48, :]

            for i in range(NSPLIT, NCH):
                nc.sync.dma_start(xx[:, i, 0, :], d_xT_r[:, i, :])
                nc.vector.tensor_mul(xx[:, i, 1, :], xx[:, i, 0, :],
                                     xx[:, i, 0, :])
            xxs = [xx[:, i, :, :] for i in range(NCH)]
            xs = [xx[:, i, 0, :] for i in range(NCH)]

            t_w1 = consts.tile([128, 12, A], bf16)
            nc.sync.dma_start(t_w1, d_w1.rearrange("(n p) a -> p n a", p=128))

            t_xN = big.tile([128, 4, T], bf16)
            d_xN_r = d_xN.rearrange("(n p) t -> p n t", p=128)
            rbp = consts.tile([128, 4, 512], bf16)
            for i in range(4):
                nc.sync.dma_start(t_xN[:, i, :], d_xN_r[:, i, :])
                if i == 1:
                    nc.sync.dma_start(
                        rbp, d_rcntb.rearrange("p (g t) -> p g t", g=4))

            # ---------------- phase-1 chunk totals ----------------
            # One PSUM accumulation group; chunk i's [x|x^2] total lands in
            # row i.  Early partial eviction of rows 0:8 unblocks the
            # first-half carries; full eviction after chunk 15.
            tot12a = colp.tile([8, 2, C], f32r)
            tot12 = colp.tile([16, 2, C], f32r)
            ps_t12 = ps_tot.tile([16, 2, C], f32, tag="tX")
            for i in range(NCH):
                oc = tb_onescols[:, 16 * i:16 * (i + 1)]
                for half in range(2):
                    nc.tensor.matmul(ps_t12[:, half, :], oc, xx[:, i, half, :],
                                     start=(i == 0), stop=(i == NCH - 1),
                                     skip_group_check=True)
                if i == 7:
                    nc.vector.tensor_copy(tot12a[:], ps_t12[0:8])
            nc.vector.tensor_copy(tot12[:], ps_t12[:])

            # ---- c_mean MLP path: project-then-scan (no transposes) ----
            Pm_sb = big.tile([128, T], f32r)
            zeros512 = consts.tile([128, 512], f32)
            nc.vector.memset(zeros512[:], 0.0)
            epsb = consts.tile([128, 1], f32)
            nc.vector.memset(epsb[:], EPS)
            def pm_scan(g):
                pm_ps = ps_col.tile([A, 512], f32,
                                    tag=("cB" if g % 2 else "cC"), name="pm_ps")
                for cb in range(4):
                    nc.tensor.matmul(pm_ps[:], t_w1[:, 4 + cb, :],
                                     t_xN[:, cb, 512 * g:512 * (g + 1)],
                                     start=(cb == 0), stop=(cb == 3))
                sl = Pm_sb[:, 512 * g:512 * (g + 1)]
                init = (0.0 if g == 0
                        else Pm_sb[:, 512 * g - 1:512 * g].bitcast(f32))
                nc.vector.tensor_tensor_scan(sl, pm_ps[:], zeros512[:],
                                             initial=init,
                                             op0=OP.add, op1=OP.add)

            def pm_rescale(g):
                sl = Pm_sb[:, 512 * g:512 * (g + 1)]
                nc.gpsimd.tensor_mul(sl, sl.bitcast(f32), rbp[:, g, :])

            # ---------------- phase 1 + MLP (lagged) ----------------
            css = [None] * NCH
            eT = colp.tile([128, NCH], f32r)
            # hpre overlays Pm_sb: Pm's group-g slice has its last read (the
            # ident matmul) right before hpre's group-g write.  Written as
            # f32r to satisfy the fp32r-rounding rule on Pm_sb's consumers.
            hpre_sb = Pm_sb
            wtris = []

            def csel_ap(i):
                k = 8 if i <= 8 else NCH
                sl = t_sutri16r[0:k, i:i + 1]
                return bass.AP(tensor=sl.tensor, offset=sl.offset,
                               ap=[[sl.ap[0][0], k], [0, 128]])

            def carry_rhs(i):
                return tot12a[:] if i <= 8 else tot12[:]

            # MLP for group g is split into 4 per-cb units spread over
            # consecutive iterations so the ACT queue never bursts.
            phs = {}
            mlp_state = {"sq_ev": None}

            def mlp_unit(g, u):
                if u == 0:
                    ph = ps_col.tile([A, 512], f32, tag="cC", name="ph")
                    phs[g] = ph
                    for cb in range(4):
                        nc.tensor.matmul(ph[:], t_w1[:, cb, :],
                                         t_xN[:, cb, 512 * g:512 * (g + 1)],
                                         start=(cb == 0), stop=False)
                    nc.tensor.matmul(ph[:], t_ident[:],
                                     Pm_sb[:, 512 * g:512 * (g + 1)],
                                     start=False, stop=False)
                ph = phs[g]
                cb = u
                if (4 * g + u) % 2 == 0:
                    ptr = ps_col.tile([128, 512], bf16, tag="cB", name="ptr")
                else:
                    ptr = ps_tot.tile([128, 512], bf16, tag="tX", name="ptr")
                src_t = css if g < 3 else csq
                for k in range(4):
                    nc.tensor.transpose(
                        ptr[:, 128 * k:128 * (k + 1)],
                        src_t[4 * g + k][:, 128 * cb:128 * (cb + 1)],
                        tb_ident[:])
                nat = natp.tile([128, 512], bf16, tag="nat")
                if g < 3:
                    # fused sqrt + PSUM->SBUF eviction (ACT)
                    mlp_state["sq_ev"] = nc.scalar.activation(
                        nat[:], ptr[:], AF.Sqrt)
                else:
                    # g3 is pre-sqrt'd in the chunk chain (pulls the ACT
                    # table switch for tanh earlier); plain DVE eviction.
                    nc.vector.tensor_copy(nat[:], ptr[:])
                nc.tensor.matmul(ph[:], t_w1[:, 8 + cb, :], nat[:],
                                 start=False, stop=(cb == 3))
                if u == 3:
                    # evict ph -> SBUF (DVE; keeps the ACT queue short)
                    nc.vector.tensor_copy(
                        hpre_sb[:, 512 * g:512 * (g + 1)], ph[:])

            var1s = [None] * NCH
            csq = [None] * NCH

            def clip1(i):
                # clip (Pool) -> bf16 tile consumed by the MLP transposes
                cs = csp_pool.tile([128, C], bf16, tag="cs")
                nc.gpsimd.tensor_scalar_max(cs[:], var1s[i][:], EPS)
                css[i] = cs
                if i >= 12:
                    # last group pre-sqrts in the chunk chain (ACT)
                    csx = csp_pool.tile([128, C], bf16, tag="csq", bufs=4)
                    sq_inst = nc.scalar.activation(csx[:], cs[:], AF.Sqrt)
                    csq[i] = csx
                    if i == NCH - 1:
                        mlp_state["sq_ev"] = sq_inst

            # MLP unit (g, u) runs at iteration 4g+5+u
            def sched_mlp(i):
                if i >= 5:
                    g, u = divmod(i - 5, 4)
                    if g < 4:
                        mlp_unit(g, u)

            for i in range(NCH):
                if 1 <= i <= 4:
                    pm_scan(i - 1)
                if 2 <= i <= 5:
                    pm_rescale(i - 2)
                s12 = ps_s.tile([128, 2, C], f32, tag="sA", name="s12")
                for half in range(2):
                    nc.tensor.matmul(s12[:, half, :], tb_triu[:],
                                     xx[:, i, half, :],
                                     start=True, stop=(i == 0),
                                     skip_group_check=True)
                    if i > 0:
                        nc.tensor.matmul(s12[:, half, :], csel_ap(i),
                                         carry_rhs(i)[:, half, :],
                                         start=False, stop=True,
                                         skip_group_check=True)
                # sq = (S1 * rcnt)^2  (ACT, PSUM->SBUF)
                sqm = tmp.tile([128, C], f32, tag="tA", bufs=3)
                nc.scalar.activation(sqm[:], s12[:, 0, :], AF.Square,
                                     scale=t_rcnt[:, i:i + 1])
                # var = S2*rcnt - sq  (DVE, PSUM+SBUF->SBUF)
                var1 = tmp.tile([128, C], f32, tag="tB", bufs=4)
                nc.vector.scalar_tensor_tensor(var1[:], s12[:, 1, :],
                                               t_rcnt[:, i:i + 1], sqm[:],
                                               op0=OP.mult, op1=OP.subtract)
                var1s[i] = var1
                clip1(i)
                sched_mlp(i)
            for i in range(NCH, 5 + 4 * 4):
                sched_mlp(i)
            last_sqrt_inst = mlp_state["sq_ev"]

            # ---- deferred tanh/scores/e (single exp-table residency) ----
            # totMA overlays the phase-1 totals (disjoint lifetimes)
            totMAa = tot12a
            totMA = tot12
            ps_tMA = ps_tot.tile([16, 2, C], f32, tag="tX")
            for g in range(4):
                h = hp.tile([A, 512], f32r, tag="h_sb")
                tanh_inst = nc.scalar.activation(
                    h[:], hpre_sb[:, 512 * g:512 * (g + 1)].bitcast(f32),
                    AF.Tanh, bias=t_b1[:])
                if g == 0:
                    add_dep_helper(tanh_inst.ins, last_sqrt_inst.ins, sync=False,
                                   reason="group exp-table ACT ops after sqrts")
                ps_sc = ps_col.tile([128, 4], f32, tag="cB")
                for k in range(4):
                    nc.tensor.matmul(
                        ps_sc[:, k:k + 1],
                        h[:, 128 * k:128 * (k + 1)].bitcast(f32),
                        t_w2[:], start=True, stop=True)
                nc.vector.tensor_add(eT[:, 4 * g:4 * g + 4], ps_sc[:],
                                     t_maskexp[:, 4 * g:4 * g + 4])
                exp_inst = nc.scalar.activation(
                    eT[:, 4 * g:4 * g + 4],
                    eT[:, 4 * g:4 * g + 4].bitcast(f32), AF.Exp)
                if g == 3:
                    last_exp_inst = exp_inst
                for ii in range(4 * g, 4 * g + 4):
                    wtri = wsp.tile([128, 128], bf16, tag=f"wtri_{ii}")
                    nc.vector.tensor_scalar_mul(
                        wtri[:], tb_triu[:],
                        eT[:, ii:ii + 1].bitcast(f32))
                    wcol = wsp.tile([128, 16], bf16, tag=f"wcol_{ii}")
                    nc.vector.tensor_scalar_mul(
                        wcol[:], tb_onescols[:, 16 * ii:16 * (ii + 1)],
                        eT[:, ii:ii + 1].bitcast(f32))
                    wtris.append(wtri)
                    for half in range(2):
                        nc.tensor.matmul(ps_tMA[:, half, :], wcol[:],
                                         xx[:, ii, half, :],
                                         start=(ii == 0), stop=(ii == NCH - 1),
                                         skip_group_check=True)


            # ---------------- phase 2 (pipelined, stage-split) ----------------
            # PSUM: cycle map tiles through sA's two slots plus the now-idle
            # totals bank (tX) for triple buffering.
            maps = [None] * NCH
            wms = [None] * NCH
            zps = [None] * NCH
            var2s = [None] * NCH
            rl2s = [None] * NCH
            wss = [None] * NCH
            first_ws = [True]

            def map_wtri(i):
                if i % 3 != 2:
                    mp = ps_s.tile([128, 2, C], f32, tag="sA", bufs=2,
                                   name="mp")
                else:
                    mp = ps_tot.tile([128, 2, C], f32, tag="tX", bufs=1,
                                     name="mp")
                for half in range(2):
                    nc.tensor.matmul(mp[:, half, :], wtris[i][:],
                                     xx[:, i, half, :],
                                     start=True, stop=(i == 0),
                                     skip_group_check=True)
                maps[i] = mp

            def map_carry(i):
                mp = maps[i]
                for half in range(2):
                    nc.tensor.matmul(mp[:, half, :], csel_ap(i),
                                     totMA[0:(8 if i <= 8 else 16), half, :],
                                     start=False, stop=True,
                                     skip_group_check=True)

            def map_mm(i):
                map_wtri(i)
                if i > 0:
                    map_carry(i)

            def stageA(i):
                mp = maps[i]
                # wm = mp * R (ACT Copy w/ scale, PSUM->SBUF, f32r)
                wm = wws_pool.tile([128, C], bf16, tag="wm")
                wm_inst = nc.scalar.activation(wm[:], mp[:, 0, :], AF.Copy,
                                               scale=R_col[:, i:i + 1])
                if first_ws[0]:
                    add_dep_helper(wm_inst.ins, last_exp_inst.ins, sync=False,
                                   reason="sqrt-table ACT ops after exp block")
                    first_ws[0] = False
                wms[i] = wm

            def stageB(i):
                # sq2 = wm^2 (DVE), var2 = ap*R - sq2 (DVE, frees PSUM slot)
                zp = tmp.tile([128, C], bf16, tag="tA2", bufs=3)
                nc.vector.tensor_mul(zp[:], wms[i][:], wms[i][:])
                zps[i] = zp
                var2 = tmp.tile([128, C], f32, tag="tB", bufs=4)
                nc.vector.scalar_tensor_tensor(var2[:], maps[i][:, 1, :],
                                               R_col[:, i:i + 1], zp[:],
                                               op0=OP.mult, op1=OP.subtract)
                var2s[i] = var2

            def stageC(i):
                # clip (Pool; ACT Relu for the tail when Pool queue drains
                # slower than the idle ACT) + sqrt with +eps bias (ACT)
                rl2 = tmp.tile([128, C], f32, tag="tC", bufs=3)
                if i >= NCH - 2:
                    nc.scalar.activation(rl2[:], var2s[i][:], AF.Relu)
                else:
                    nc.gpsimd.tensor_scalar_max(rl2[:], var2s[i][:], 0.0)
                ws = wws_pool.tile([128, C], bf16, tag="ws", bufs=4)
                nc.scalar.activation(ws[:], rl2[:], AF.Sqrt, bias=epsb[:])
                rl2s[i], wss[i] = rl2, ws

            def finals(k):
                nc.tensor.matmul(ps_fm[:], tb_finalw[:, k:k + 1], wms[k][:],
                                 start=(k == 0), stop=(k == NCH - 1))
                nc.tensor.matmul(ps_fs[:], tb_finalw[:, k:k + 1], wss[k][:],
                                 start=(k == 0), stop=(k == NCH - 1))

            # chunk 0 + chunk 1's wtri matmuls first, then the E->R block
            # (overlaps with the PE stream); chunk 1's carry follows in-loop.
            map_mm(0)
            map_wtri(1)
            map_wtri(2)

            ps_E = ps_col.tile([128, NCH], f32, tag="cB")
            nc.tensor.matmul(ps_E[:], t_triu[:], eT[:], start=True, stop=False)
            ps_et = ps_col.tile([16, 1], f32, tag="cC")
            nc.tensor.matmul(ps_et[:], eT[:].bitcast(f32), t_onesc[:],
                             start=True, stop=True)
            etot = colp.tile([16, 1], f32)
            nc.vector.tensor_copy(etot[:], ps_et[:])
            lhs_bc = bass.AP(tensor=etot.tensor, offset=etot.offset,
                             ap=[[etot[:].ap[0][0], 16], [0, 128]])
            nc.tensor.matmul(ps_E[:], lhs_bc, t_sutri16[:], start=False,
                             stop=True)
            R_col = colp.tile([128, NCH], f32)
            nc.vector.reciprocal(R_col[:], ps_E[:])
            # phase-2 totals eviction: halves split across DVE and ACT so
            # neither delays R/wm_0; per-half carries unblock independently
            nc.vector.tensor_copy(totMA[:, 0, :], ps_tMA[:, 0, :])
            nc.scalar.copy(totMA[:, 1, :], ps_tMA[:, 1, :])

            ps_fm = ps_col.tile([1, C], f32, tag="cB")
            ps_fs = ps_col.tile([1, C], f32, tag="cC")
            for it in range(1, NCH + FIN_LAG + 1):
                if 0 <= it - 1 < NCH:
                    stageA(it - 1)
                if it in (1, 2):
                    map_carry(it)
                elif it < NCH:
                    map_mm(it)
                if 0 <= it - 2 < NCH:
                    stageB(it - 2)
                if 0 <= it - 3 < NCH:
                    stageC(it - 3)
                k = it - FIN_LAG - 1
                if 0 <= k:
                    finals(k)

            out_sb = colp.tile([1, 2 * C], f32)
            nc.vector.tensor_copy(out_sb[:, 0:C], ps_fm[:])
            nc.sync.dma_start(d_out[:, 0:C], out_sb[:, 0:C])
            nc.scalar.copy(out_sb[:, C:2 * C], ps_fs[:])
            nc.sync.dma_start(d_out[:, C:2 * C], out_sb[:, C:2 * C])

    nc.compile()
    return nc


def _host_inputs(x, lengths, w1, b1, w2, b2):
    """Per-core DRAM input maps."""
    x = np.asarray(x, np.float32)
    lengths = np.asarray(lengths)
    w1 = np.asarray(w1, np.float32)
    b1 = np.asarray(b1, np.float32)
    w2 = np.asarray(w2, np.float32)
    b2 = np.asarray(b2, np.float32)

    triu = np.triu(np.ones((128, 128), np.float32))
    ident = np.eye(128, dtype=np.float32)
    onescols = np.zeros((128, 16 * NCH), np.float32)
    for i in range(NCH):
        onescols[:, 16 * i + i] = 1.0
    sutri16 = np.ascontiguousarray(np.triu(np.ones((NCH, NCH), np.float32), 1))
    sutri48 = np.zeros((48, NCH), np.float32)
    sutri48[0:NCH] = sutri16
    sutri48[32:48] = sutri16
    tt = np.arange(T)

    maps = []
    for b in range(B):
        L = int(lengths[b])
        rcnt = (1.0 / np.minimum(tt + 1, max(L, 1))).astype(np.float32)
        maskexp = (float(b2[0]) + np.where(tt < L, 0.0, NEG)).astype(np.float32)
        finalw = np.where(tt < L, 1.0 / max(L, 1), 0.0).astype(np.float32)
        blob = np.zeros((128, 595), np.float32)
        blob[:, 0:128] = triu
        blob[:, 128:384] = onescols
        blob[:, 384:512] = ident
        blob[0:48, 512:528] = sutri48
        blob[0:16, 528:544] = sutri16
        blob[:, 544] = 1.0
        blob[:, 545:561] = rcnt.reshape(NCH, 128).T
        blob[:, 561:577] = maskexp.reshape(NCH, 128).T
        blob[:, 577:593] = finalw.reshape(NCH, 128).T
        blob[:, 593] = w2[:, 0]
        blob[:, 594] = b1
        bb = np.zeros((128, 528), np.float32)
        bb[:, 0:128] = triu
        bb[:, 128:384] = onescols
        bb[:, 384:512] = ident
        bb[:, 512:528] = finalw.reshape(NCH, 128).T
        maps.append({
            "xT": np.ascontiguousarray(x[b].T).astype(ml_dtypes.bfloat16),
            "xN": np.ascontiguousarray(x[b]).astype(ml_dtypes.bfloat16),
            "w1": w1.astype(ml_dtypes.bfloat16),
            "bblob": bb.astype(ml_dtypes.bfloat16),
            "rcntb": np.ascontiguousarray(
                np.broadcast_to(rcnt[None, :], (128, T))).astype(
                    ml_dtypes.bfloat16),
            "cblob": blob,
        })
    return maps


def kernel(x, lengths, w1, b1, w2, b2):
    from concourse.bass_utils import run_bass_kernel_spmd

    if "nc" not in _CACHE:
        _CACHE["nc"] = _build()
    nc = _CACHE["nc"]
    maps = _host_inputs(x, lengths, w1, b1, w2, b2)
    res = run_bass_kernel_spmd(nc, maps, list(range(B))).results
    out = np.stack([res[b]["out"][0] for b in range(B)], axis=0)
    return out.astype(np.float32)
